# revision 7
# baseline (speedup 1.0000x reference)
"""Trainium2 Bass kernel for nn_CrossAttention (B=4, N=M=2048, DIM=1024, H=16, D=64).

Sharding: batch x head-group over 8 cores. Core c handles batch b = c//2 and
heads hgrp = c%2 (8 heads, a contiguous 512-wide slice of the hidden dim).
Each core computes q/k/v projections for its heads, flash-style attention in
S^T layout (keys on partitions), and a *partial* output projection over its
512 hidden dims. The host sums the two partials per batch and adds the output
bias (the only cross-core reduction).

Device layouts (chosen so every matmul has its contraction dim on partitions):
  xT/cT   [E=1024, N=2048]   (host-transposed, bf16)
  qT/kT   [512, 2048]        d-on-partitions, produced by lhsT=W^T, rhs=xT
  v_aug   [m, head, 65]      keys-on-partitions; col 64 == 1.0 so the PV
                             matmul also yields the softmax denominator
  S^T     [m=128, n=1024]    PSUM; exp on ScalarE (scale=1/8 fused)
  O^T     [65, 512] PSUM     partition 64 = sum_m exp(S); normalization via a
                             K=1 broadcast matmul + reciprocal + multiply
"""

import numpy as np
import ml_dtypes

import concourse.bass as bass
import concourse.mybir as mybir
import concourse.tile as tile
from concourse import bacc
from concourse.bass_utils import run_bass_kernel_spmd

N_CORES = 8
B, N, M, DIM = 4, 2048, 2048, 1024
HEADS, HD = 16, 64            # total heads, head dim
HPC = 8                       # heads per core
CW = HPC * HD                 # per-core hidden width = 512
EC = DIM // 128               # 8 contraction chunks of 128
DC = CW // 128                # 4 chunks of the per-core q/k dims
MC = M // 128                 # 16 key chunks
NB = N // 512                 # 4 query 512-blocks
SCALE = HD ** -0.5            # 0.125

F32 = mybir.dt.float32
F32R = mybir.dt.float32r
BF16 = mybir.dt.bfloat16
nbf = ml_dtypes.bfloat16


def _build_nc():
    nc = bacc.Bacc("TRN2", target_bir_lowering=False, debug=False,
                   num_devices=N_CORES)

    xT = nc.dram_tensor("xT", [DIM, N], BF16, kind="ExternalInput")
    cT = nc.dram_tensor("cT", [DIM, M], BF16, kind="ExternalInput")
    wq = nc.dram_tensor("wq", [DIM, CW], BF16, kind="ExternalInput")
    wk = nc.dram_tensor("wk", [DIM, CW], BF16, kind="ExternalInput")
    wv = nc.dram_tensor("wv", [DIM, CW], BF16, kind="ExternalInput")
    wo = nc.dram_tensor("wo", [CW, DIM], BF16, kind="ExternalInput")
    bq = nc.dram_tensor("bq", [CW], F32, kind="ExternalInput")
    bk = nc.dram_tensor("bk", [CW], F32, kind="ExternalInput")
    bv = nc.dram_tensor("bv", [CW], F32, kind="ExternalInput")
    out = nc.dram_tensor("out", [N, DIM], F32, kind="ExternalOutput")

    with tile.TileContext(nc) as tc:
        with (
            tc.tile_pool(name="persist", bufs=1) as pp,
            tc.tile_pool(name="work", bufs=2) as wp,
            tc.tile_pool(name="ps_small", bufs=2, space="PSUM") as ps_s,
            tc.tile_pool(name="ps_big", bufs=3, space="PSUM") as ps_b,
        ):
            # ---- persistent SBUF tensors -------------------------------
            wq_sb = pp.tile([128, EC, CW], BF16, tag="wq")
            wk_sb = pp.tile([128, EC, CW], BF16, tag="wk")
            wv_sb = pp.tile([128, EC, CW], BF16, tag="wv")
            wo_sb = pp.tile([128, DC, DIM], BF16, tag="wo")
            qT_sb = pp.tile([128, DC, N], BF16, tag="qT")
            kT_sb = pp.tile([128, DC, M], BF16, tag="kT")
            v_sb = pp.tile([128, MC, HPC, 128], BF16, tag="v")
            pT_sb = pp.tile([128, MC, 1024], BF16, tag="pT")
            st_sb = pp.tile([128, DC, N], BF16, tag="stacked")
            ones_sb = pp.tile([128, 128], F32, tag="ones")
            bq_sb = pp.tile([128, DC], F32, tag="bq")
            bk_sb = pp.tile([128, DC], F32, tag="bk")
            bv1_sb = pp.tile([1, CW], F32, tag="bv1")
            bvb_sb = pp.tile([128, CW], F32, tag="bvb")

            # weights/biases go on the gpsimd DMA queue so activation-chunk
            # loads on the sync queue run in parallel with them
            nc.gpsimd.dma_start(wk_sb[:], wk.ap().rearrange("(e p) c -> p e c", p=128))
            nc.gpsimd.dma_start(bk_sb[:], bk.ap().rearrange("(d p) -> p d", p=128))
            nc.gpsimd.dma_start(bv1_sb[:], bv.ap().rearrange("(a c) -> a c", a=1))
            nc.gpsimd.dma_start(wv_sb[:], wv.ap().rearrange("(e p) c -> p e c", p=128))
            nc.gpsimd.dma_start(wq_sb[:], wq.ap().rearrange("(e p) c -> p e c", p=128))
            nc.gpsimd.dma_start(bq_sb[:], bq.ap().rearrange("(d p) -> p d", p=128))
            nc.gpsimd.dma_start(wo_sb[:], wo.ap().rearrange("(e p) c -> p e c", p=128))
            nc.vector.memset(ones_sb[:], 1.0)
            nc.vector.memset(v_sb[:, :, :, 0:HD], 1.0)

            # broadcast bv across partitions: [1,512] -> [128,512] via K=1 matmul
            bvp = ps_s.tile([128, 512], F32, tag="po")
            nc.tensor.matmul(
                bvp[:],
                ones_sb[0:1, 0:128],
                bv1_sb[0:1, :],
                start=True, stop=True,
            )
            nc.vector.tensor_copy(out=bvb_sb[:], in_=bvp[:])

            def load_chunk(src, nb):
                # split in two DMAs for finer dependency granularity
                chunk = wp.tile([128, EC, 512], BF16, tag="src_chunk")
                rsrc = src.ap().rearrange("(e p) n -> p e n", p=128)
                for half in range(2):
                    nc.sync.dma_start(
                        chunk[:, half * 4:(half + 1) * 4, :],
                        rsrc[:, half * 4:(half + 1) * 4,
                             nb * 512:(nb + 1) * 512],
                    )
                return chunk

            def attention(h, nb2):
                hp, dc = h % 2, h // 2
                dsl = slice(hp * 64, hp * 64 + 64)
                for mc in range(MC):
                    stp = ps_b.tile([128, 1024], F32, tag="st")
                    for hf in range(2):
                        nc.tensor.matmul(
                            stp[:, hf * 512:(hf + 1) * 512],
                            kT_sb[dsl, dc, mc * 128:(mc + 1) * 128],
                            qT_sb[dsl, dc,
                                  nb2 * 1024 + hf * 512:
                                  nb2 * 1024 + (hf + 1) * 512],
                            start=True, stop=True,
                        )
                    nc.scalar.activation(
                        pT_sb[:, mc, :], stp[:],
                        mybir.ActivationFunctionType.Exp,
                        scale=SCALE,
                    )
                for ns in range(2):       # 512-wide blocks within nb2
                    # lhsT = [ones*64 | v_h]: partitions 0-63 of the result
                    # all equal sum_m exp(S) (free in-matmul broadcast of
                    # the softmax denominator), partitions 64-127 are O^T.
                    po = ps_s.tile([128, 512], F32, tag="po")
                    for mc in range(MC):
                        nc.tensor.matmul(
                            po[:],
                            v_sb[:, mc, h, :],
                            pT_sb[:, mc, ns * 512:(ns + 1) * 512],
                            start=(mc == 0), stop=(mc == MC - 1),
                        )
                    rbc = wp.tile([128, 512], F32, tag="rbc")
                    nc.vector.reciprocal_approx_fast(
                        out=rbc[0:64, :], in_=po[0:64, :])
                    nsl = slice(nb2 * 1024 + ns * 512,
                                nb2 * 1024 + (ns + 1) * 512)
                    if hp == 0:
                        nc.vector.tensor_tensor(
                            out=st_sb[0:64, dc, nsl],
                            in0=po[64:128, :], in1=rbc[0:64, :],
                            op=mybir.AluOpType.mult,
                        )
                    else:
                        tmp = wp.tile([64, 512], BF16, tag="otmp")
                        nc.vector.tensor_tensor(
                            out=tmp[:], in0=po[64:128, :], in1=rbc[0:64, :],
                            op=mybir.AluOpType.mult,
                        )
                        nc.sync.dma_start(st_sb[64:128, dc, nsl], tmp[:])

            def out_proj(nb2):
                for nck in range(nb2 * 8, (nb2 + 1) * 8):
                    for jb in range(2):
                        acc = ps_s.tile([128, 512], F32, tag="po")
                        for cc in range(DC):
                            nc.tensor.matmul(
                                acc[:],
                                st_sb[:, cc, nck * 128:(nck + 1) * 128],
                                wo_sb[:, cc, jb * 512:(jb + 1) * 512],
                                start=(cc == 0), stop=(cc == DC - 1),
                            )
                        ot = wp.tile([128, 512], F32, tag="out")
                        nc.vector.tensor_copy(out=ot[:], in_=acc[:])
                        nc.sync.dma_start(
                            out.ap()[nck * 128:(nck + 1) * 128,
                                     jb * 512:(jb + 1) * 512],
                            ot[:],
                        )

            # ---- phase C: kT + v projections from context -------------
            for nb in range(NB):
                chunk = load_chunk(cT, nb)
                for dc in range(DC):
                    acc = ps_s.tile([128, 512], F32, tag="po")
                    for ec in range(EC):
                        nc.tensor.matmul(
                            acc[:],
                            wk_sb[:, ec, dc * 128:(dc + 1) * 128],
                            chunk[:, ec, :],
                            start=(ec == 0), stop=(ec == EC - 1),
                        )
                    nc.vector.tensor_scalar_add(
                        kT_sb[:, dc, nb * 512:(nb + 1) * 512],
                        acc[:],
                        bk_sb[:, dc:dc + 1],
                    )
                for mi in range(4):
                    mc = nb * 4 + mi
                    accv = ps_s.tile([128, 512], F32, tag="po")
                    for ec in range(EC):
                        nc.tensor.matmul(
                            accv[:],
                            chunk[:, ec, mi * 128:(mi + 1) * 128],
                            wv_sb[:, ec, :],
                            start=(ec == 0), stop=(ec == EC - 1),
                        )
                    nc.vector.tensor_tensor(
                        out=v_sb[:, mc, :, HD:128],
                        in0=accv[:].rearrange("p (h d) -> p h d", h=HPC),
                        in1=bvb_sb[:].rearrange("p (h d) -> p h d", h=HPC),
                        op=mybir.AluOpType.add,
                    )

            # ---- phase X+A: qT projection (dc-major) interleaved with
            # attention; heads 2dc,2dc+1 start right after qT[dc] is done.
            for dc in range(DC):
                for nb in range(NB):
                    chunk = load_chunk(xT, nb)
                    acc = ps_s.tile([128, 512], F32, tag="po")
                    for ec in range(EC):
                        nc.tensor.matmul(
                            acc[:],
                            wq_sb[:, ec, dc * 128:(dc + 1) * 128],
                            chunk[:, ec, :],
                            start=(ec == 0), stop=(ec == EC - 1),
                        )
                    nc.vector.tensor_scalar_add(
                        qT_sb[:, dc, nb * 512:(nb + 1) * 512],
                        acc[:],
                        bq_sb[:, dc:dc + 1],
                    )
                for hp in range(2):
                    h = 2 * dc + hp
                    for nb2 in range(2):
                        attention(h, nb2)
                        if h == HPC - 1 and nb2 == 0:
                            out_proj(0)
            out_proj(1)
    nc.compile()
    return nc


_NC_CACHE = None


def _get_nc():
    global _NC_CACHE
    if _NC_CACHE is None:
        _NC_CACHE = _build_nc()
    return _NC_CACHE


def make_in_maps(x, context, Wq, bq, Wk, bk, Wv, bv, Wo, bo):
    """Host-side sharding: per-core transposed bf16 operand prep."""
    in_maps = []
    for c in range(N_CORES):
        b, hg = c // 2, c % 2
        cs = slice(hg * CW, hg * CW + CW)
        in_maps.append({
            "xT": np.ascontiguousarray(x[b].T).astype(nbf),
            "cT": np.ascontiguousarray(context[b].T).astype(nbf),
            "wq": np.ascontiguousarray(Wq[cs].T).astype(nbf),
            "wk": np.ascontiguousarray(Wk[cs].T).astype(nbf),
            "wv": np.ascontiguousarray(Wv[cs].T).astype(nbf),
            "wo": np.ascontiguousarray(Wo[:, cs].T).astype(nbf),
            "bq": np.ascontiguousarray(bq[cs]).astype(np.float32),
            "bk": np.ascontiguousarray(bk[cs]).astype(np.float32),
            "bv": np.ascontiguousarray(bv[cs]).astype(np.float32),
        })
    return in_maps


def gather(results, bo):
    """Host-side unshard: sum the two head-group partials per batch, add bo."""
    out = np.empty((B, N, DIM), np.float32)
    for b in range(B):
        out[b] = results[2 * b]["out"] + results[2 * b + 1]["out"]
    out += np.asarray(bo, np.float32)[None, None, :]
    return out


def kernel(x, context, Wq, bq, Wk, bk, Wv, bv, Wo, bo):
    nc = _get_nc()
    in_maps = make_in_maps(x, context, Wq, bq, Wk, bk, Wv, bv, Wo, bo)
    res = run_bass_kernel_spmd(nc, in_maps, list(range(N_CORES)))
    return gather(res.results, bo)


# revision 9
# speedup vs baseline: 1.0253x; 1.0253x over previous
"""Trainium2 Bass kernel for nn_CrossAttention (B=4, N=M=2048, DIM=1024, H=16, D=64).

Sharding: batch x head-group over 8 cores. Core c handles batch b = c//2 and
heads hgrp = c%2 (8 heads, a contiguous 512-wide slice of the hidden dim).
Each core computes q/k/v projections for its heads, flash-style attention in
S^T layout (keys on partitions), and a *partial* output projection over its
512 hidden dims. The host sums the two partials per batch and adds the output
bias (the only cross-core reduction).

Device layouts (chosen so every matmul has its contraction dim on partitions):
  xT/cT   [E=1024, N=2048]   (host-transposed, bf16)
  qT/kT   [512, 2048]        d-on-partitions, produced by lhsT=W^T, rhs=xT
  v_aug   [m, head, 65]      keys-on-partitions; col 64 == 1.0 so the PV
                             matmul also yields the softmax denominator
  S^T     [m=128, n=1024]    PSUM; exp on ScalarE (scale=1/8 fused)
  O^T     [65, 512] PSUM     partition 64 = sum_m exp(S); normalization via a
                             K=1 broadcast matmul + reciprocal + multiply
"""

import numpy as np
import ml_dtypes

import concourse.bass as bass
import concourse.mybir as mybir
import concourse.tile as tile
from concourse import bacc
from concourse.bass_utils import run_bass_kernel_spmd

N_CORES = 8
B, N, M, DIM = 4, 2048, 2048, 1024
HEADS, HD = 16, 64            # total heads, head dim
HPC = 8                       # heads per core
CW = HPC * HD                 # per-core hidden width = 512
EC = DIM // 128               # 8 contraction chunks of 128
DC = CW // 128                # 4 chunks of the per-core q/k dims
MC = M // 128                 # 16 key chunks
NB = N // 512                 # 4 query 512-blocks
SCALE = HD ** -0.5            # 0.125

F32 = mybir.dt.float32
F32R = mybir.dt.float32r
BF16 = mybir.dt.bfloat16
nbf = ml_dtypes.bfloat16


def _build_nc():
    nc = bacc.Bacc("TRN2", target_bir_lowering=False, debug=False,
                   num_devices=N_CORES)

    xT = nc.dram_tensor("xT", [DIM, N], BF16, kind="ExternalInput")
    cT = nc.dram_tensor("cT", [DIM, M], BF16, kind="ExternalInput")
    wq = nc.dram_tensor("wq", [DIM, CW], BF16, kind="ExternalInput")
    wk = nc.dram_tensor("wk", [DIM, CW], BF16, kind="ExternalInput")
    wv = nc.dram_tensor("wv", [DIM, CW], BF16, kind="ExternalInput")
    wo = nc.dram_tensor("wo", [CW, DIM], BF16, kind="ExternalInput")
    bq = nc.dram_tensor("bq", [CW], F32, kind="ExternalInput")
    bk = nc.dram_tensor("bk", [CW], F32, kind="ExternalInput")
    bv = nc.dram_tensor("bv", [CW], F32, kind="ExternalInput")
    out = nc.dram_tensor("out", [N, DIM], F32, kind="ExternalOutput")

    with tile.TileContext(nc) as tc:
        with (
            tc.tile_pool(name="persist", bufs=1) as pp,
            tc.tile_pool(name="work", bufs=2) as wp,
            tc.tile_pool(name="ps_small", bufs=4, space="PSUM") as ps_s,
            tc.tile_pool(name="ps_big", bufs=2, space="PSUM") as ps_b,
        ):
            # ---- persistent SBUF tensors -------------------------------
            wq_sb = pp.tile([128, EC, CW], BF16, tag="wq")
            wk_sb = pp.tile([128, EC, CW], BF16, tag="wk")
            wv_sb = pp.tile([128, EC, CW], BF16, tag="wv")
            wo_sb = pp.tile([128, DC, DIM], BF16, tag="wo")
            qT_sb = pp.tile([128, DC, N], BF16, tag="qT")
            kT_sb = pp.tile([128, DC, M], BF16, tag="kT")
            v_sb = pp.tile([128, MC, HPC, 128], BF16, tag="v")
            pT_sb = pp.tile([128, MC, 1024], BF16, tag="pT")
            st_sb = pp.tile([128, DC, N], BF16, tag="stacked")
            ones_sb = pp.tile([128, 128], F32, tag="ones")
            bq_sb = pp.tile([128, DC], F32, tag="bq")
            bk_sb = pp.tile([128, DC], F32, tag="bk")
            bv1_sb = pp.tile([1, CW], F32, tag="bv1")
            bvb_sb = pp.tile([128, CW], F32, tag="bvb")

            # weights/biases go on the gpsimd DMA queue so activation-chunk
            # loads on the sync queue run in parallel with them
            nc.gpsimd.dma_start(wk_sb[:], wk.ap().rearrange("(e p) c -> p e c", p=128))
            nc.gpsimd.dma_start(bk_sb[:], bk.ap().rearrange("(d p) -> p d", p=128))
            nc.gpsimd.dma_start(bv1_sb[:], bv.ap().rearrange("(a c) -> a c", a=1))
            nc.gpsimd.dma_start(wv_sb[:], wv.ap().rearrange("(e p) c -> p e c", p=128))
            nc.gpsimd.dma_start(wq_sb[:], wq.ap().rearrange("(e p) c -> p e c", p=128))
            nc.gpsimd.dma_start(bq_sb[:], bq.ap().rearrange("(d p) -> p d", p=128))
            nc.gpsimd.dma_start(wo_sb[:], wo.ap().rearrange("(e p) c -> p e c", p=128))
            nc.vector.memset(ones_sb[:], 1.0)
            nc.vector.memset(v_sb[:, :, :, 0:HD], 1.0)

            # broadcast bv across partitions: [1,512] -> [128,512] via K=1 matmul
            bvp = ps_s.tile([128, 512], F32, tag="po")
            nc.tensor.matmul(
                bvp[:],
                ones_sb[0:1, 0:128],
                bv1_sb[0:1, :],
                start=True, stop=True,
            )
            nc.vector.tensor_copy(out=bvb_sb[:], in_=bvp[:])

            def load_chunk(src, nb):
                # split in two DMAs for finer dependency granularity
                chunk = wp.tile([128, EC, 512], BF16, tag="src_chunk")
                rsrc = src.ap().rearrange("(e p) n -> p e n", p=128)
                for half in range(2):
                    nc.sync.dma_start(
                        chunk[:, half * 4:(half + 1) * 4, :],
                        rsrc[:, half * 4:(half + 1) * 4,
                             nb * 512:(nb + 1) * 512],
                    )
                return chunk

            def attention(h, nb2):
                hp, dc = h % 2, h // 2
                dsl = slice(hp * 64, hp * 64 + 64)
                for mc in range(MC):
                    stp = ps_b.tile([128, 1024], F32, tag="st")
                    for hf in range(2):
                        nc.tensor.matmul(
                            stp[:, hf * 512:(hf + 1) * 512],
                            kT_sb[dsl, dc, mc * 128:(mc + 1) * 128],
                            qT_sb[dsl, dc,
                                  nb2 * 1024 + hf * 512:
                                  nb2 * 1024 + (hf + 1) * 512],
                            start=True, stop=True,
                        )
                    nc.scalar.activation(
                        pT_sb[:, mc, :], stp[:],
                        mybir.ActivationFunctionType.Exp,
                        scale=SCALE,
                    )
                for ns in range(2):       # 512-wide blocks within nb2
                    # lhsT = [ones*64 | v_h]: partitions 0-63 of the result
                    # all equal sum_m exp(S) (free in-matmul broadcast of
                    # the softmax denominator), partitions 64-127 are O^T.
                    po = ps_s.tile([128, 512], F32, tag="po")
                    for mc in range(MC):
                        nc.tensor.matmul(
                            po[:],
                            v_sb[:, mc, h, :],
                            pT_sb[:, mc, ns * 512:(ns + 1) * 512],
                            start=(mc == 0), stop=(mc == MC - 1),
                        )
                    rbc = wp.tile([128, 512], F32, tag="rbc")
                    nc.vector.reciprocal_approx_fast(
                        out=rbc[0:64, :], in_=po[0:64, :])
                    nsl = slice(nb2 * 1024 + ns * 512,
                                nb2 * 1024 + (ns + 1) * 512)
                    if hp == 0:
                        nc.vector.tensor_tensor(
                            out=st_sb[0:64, dc, nsl],
                            in0=po[64:128, :], in1=rbc[0:64, :],
                            op=mybir.AluOpType.mult,
                        )
                    else:
                        tmp = wp.tile([64, 512], BF16, tag="otmp")
                        nc.vector.tensor_tensor(
                            out=tmp[:], in0=po[64:128, :], in1=rbc[0:64, :],
                            op=mybir.AluOpType.mult,
                        )
                        nc.sync.dma_start(st_sb[64:128, dc, nsl], tmp[:])

            def out_proj(nb2):
                for nck in range(nb2 * 8, (nb2 + 1) * 8):
                    for jb in range(2):
                        acc = ps_s.tile([128, 512], F32, tag="po")
                        for cc in range(DC):
                            nc.tensor.matmul(
                                acc[:],
                                st_sb[:, cc, nck * 128:(nck + 1) * 128],
                                wo_sb[:, cc, jb * 512:(jb + 1) * 512],
                                start=(cc == 0), stop=(cc == DC - 1),
                            )
                        ot = wp.tile([128, 512], F32, tag="out")
                        nc.vector.tensor_copy(out=ot[:], in_=acc[:])
                        nc.sync.dma_start(
                            out.ap()[nck * 128:(nck + 1) * 128,
                                     jb * 512:(jb + 1) * 512],
                            ot[:],
                        )

            # ---- phase C: kT + v projections from context -------------
            for nb in range(NB):
                chunk = load_chunk(cT, nb)
                for dc in range(DC):
                    acc = ps_s.tile([128, 512], F32, tag="po")
                    for ec in range(EC):
                        nc.tensor.matmul(
                            acc[:],
                            wk_sb[:, ec, dc * 128:(dc + 1) * 128],
                            chunk[:, ec, :],
                            start=(ec == 0), stop=(ec == EC - 1),
                        )
                    nc.vector.tensor_scalar_add(
                        kT_sb[:, dc, nb * 512:(nb + 1) * 512],
                        acc[:],
                        bk_sb[:, dc:dc + 1],
                    )
                for mi in range(4):
                    mc = nb * 4 + mi
                    accv = ps_s.tile([128, 512], F32, tag="po")
                    for ec in range(EC):
                        nc.tensor.matmul(
                            accv[:],
                            chunk[:, ec, mi * 128:(mi + 1) * 128],
                            wv_sb[:, ec, :],
                            start=(ec == 0), stop=(ec == EC - 1),
                        )
                    nc.vector.tensor_tensor(
                        out=v_sb[:, mc, :, HD:128],
                        in0=accv[:].rearrange("p (h d) -> p h d", h=HPC),
                        in1=bvb_sb[:].rearrange("p (h d) -> p h d", h=HPC),
                        op=mybir.AluOpType.add,
                    )

            # ---- phase X+A: qT projection (dc-major) interleaved with
            # attention; qT[dc+1] is emitted early so it overlaps dc's
            # attention and the next heads start without a boundary stall.
            def qt_proj(dc):
                for nb in range(NB):
                    chunk = load_chunk(xT, nb)
                    acc = ps_s.tile([128, 512], F32, tag="po")
                    for ec in range(EC):
                        nc.tensor.matmul(
                            acc[:],
                            wq_sb[:, ec, dc * 128:(dc + 1) * 128],
                            chunk[:, ec, :],
                            start=(ec == 0), stop=(ec == EC - 1),
                        )
                    nc.vector.tensor_scalar_add(
                        qT_sb[:, dc, nb * 512:(nb + 1) * 512],
                        acc[:],
                        bq_sb[:, dc:dc + 1],
                    )

            qt_proj(0)
            for dc in range(DC):
                attention(2 * dc, 0)
                if dc + 1 < DC:
                    qt_proj(dc + 1)
                attention(2 * dc, 1)
                attention(2 * dc + 1, 0)
                attention(2 * dc + 1, 1)
                if dc == DC - 1:
                    out_proj(0)
            out_proj(1)
    nc.compile()
    return nc


_NC_CACHE = None


def _get_nc():
    global _NC_CACHE
    if _NC_CACHE is None:
        _NC_CACHE = _build_nc()
    return _NC_CACHE


def make_in_maps(x, context, Wq, bq, Wk, bk, Wv, bv, Wo, bo):
    """Host-side sharding: per-core transposed bf16 operand prep."""
    in_maps = []
    for c in range(N_CORES):
        b, hg = c // 2, c % 2
        cs = slice(hg * CW, hg * CW + CW)
        in_maps.append({
            "xT": np.ascontiguousarray(x[b].T).astype(nbf),
            "cT": np.ascontiguousarray(context[b].T).astype(nbf),
            "wq": np.ascontiguousarray(Wq[cs].T).astype(nbf),
            "wk": np.ascontiguousarray(Wk[cs].T).astype(nbf),
            "wv": np.ascontiguousarray(Wv[cs].T).astype(nbf),
            "wo": np.ascontiguousarray(Wo[:, cs].T).astype(nbf),
            "bq": np.ascontiguousarray(bq[cs]).astype(np.float32),
            "bk": np.ascontiguousarray(bk[cs]).astype(np.float32),
            "bv": np.ascontiguousarray(bv[cs]).astype(np.float32),
        })
    return in_maps


def gather(results, bo):
    """Host-side unshard: sum the two head-group partials per batch, add bo."""
    out = np.empty((B, N, DIM), np.float32)
    for b in range(B):
        out[b] = results[2 * b]["out"] + results[2 * b + 1]["out"]
    out += np.asarray(bo, np.float32)[None, None, :]
    return out


def kernel(x, context, Wq, bq, Wk, bk, Wv, bv, Wo, bo):
    nc = _get_nc()
    in_maps = make_in_maps(x, context, Wq, bq, Wk, bk, Wv, bv, Wo, bo)
    res = run_bass_kernel_spmd(nc, in_maps, list(range(N_CORES)))
    return gather(res.results, bo)
